# revision 19
# baseline (speedup 1.0000x reference)
"""GCN-GRU cell fused Trainium2 kernel (8-core data parallel), v3.

Math (per batch b):
    A = d * (adj+I).T * d,  d = rowsum(adj+I)^-0.5
    conc1 = [input, hidden]                (N, 65)
    sig   = sigmoid(A @ conc1 @ W1 + b1)   (N, 128)  node-major flat
    r, u  = first/second half of flat(sig) -> pseudo-node split
    rh    = r * hidden_flat
    c     = tanh(A @ [input, rh] @ W2 + b2)
    out   = u * hidden_flat + (1-u) * c

v3 structure:
  - r/u pseudo-node split -> GCN1 runs in column halves: the r-half
    (A cols 0:1024) feeds W1 r-groups + x2 assembly while the u-half
    passes and GCN2 keep the PE busy; sigmoid latency never paces PE.
  - GCN1-r is ONE joint tp-major pass over all 4 batch-pairs x 2 chunks
    (8 psum banks live): A is consumed at ~supply rate, so the initial
    HBM-limited window has no PE stalls once started.
  - Single psum tag "big" = 8 x 1-bank ring for big chunks, W1 halves
    and W2 units: 8-deep reuse slack decouples activation pacing.
  - Full-width [128] psum drains write BOTH batch planes (garbage rows
    are nulled by zero rows in the packed W tiles); axin/ones rows are
    patched by tiny 2-row DMAs (removes v1's ~4MB zero-padding DMA).
  - One priority DMA ring (sync) carries A halves + x1 in consumption
    order, then patches, then hrm - ring FIFO guarantees A is never
    contended during the critical first ~19us (DMA pool saturates at
    ~283 GB/s aggregate).
  - GCN2 chunk-major with per-column-half patches; W2 in batch-halves
    (hb0 between GCN2 p1/p2, hb1 at the end) for a short store tail.
"""

import numpy as np
import ml_dtypes
from contextlib import ExitStack

import concourse.bacc as bacc
import concourse.mybir as mybir
import concourse.tile as tile
from concourse.bass import ts, ds
from concourse.bass_utils import run_bass_kernel_spmd

P = 128
N = 2048
B = 64
H = 64
NCORES = 8
BL = B // NCORES          # 8 batches per core
KT = N // P               # 16 contraction tiles
NT = KT // 2              # 8 (pair-tiles / half-node groups)
CH = N // 512             # 4 output chunks of 512
F32 = mybir.dt.float32
BF16 = mybir.dt.bfloat16
FP8 = mybir.dt.float8e4
SIG = mybir.ActivationFunctionType.Sigmoid
TANH = mybir.ActivationFunctionType.Tanh
DR = mybir.MatmulPerfMode.DoubleRow

_CACHE = {}


def _build():
    nc = bacc.Bacc("TRN2", target_bir_lowering=False)

    a_d = nc.dram_tensor("a", [P, 2, KT, 1024], FP8, kind="ExternalInput")
    x1_d = nc.dram_tensor("x1", [P, KT * BL * H], FP8, kind="ExternalInput")
    hrm_d = nc.dram_tensor("hrm", [P, NT, BL, 2 * H], BF16, kind="ExternalInput")
    axe_d = nc.dram_tensor("axe", [2, 4, N], BF16, kind="ExternalInput")
    axo_d = nc.dram_tensor("axo", [2, 4, N], BF16, kind="ExternalInput")
    w1_d = nc.dram_tensor("w1", [P, 2, 2 * H], BF16, kind="ExternalInput")
    w2_d = nc.dram_tensor("w2", [P, 2, H], BF16, kind="ExternalInput")
    out_d = nc.dram_tensor("out", [P, NT, 2, 4, 2 * H], BF16, kind="ExternalOutput")

    out_ap = out_d.ap()

    with tile.TileContext(nc) as tc, ExitStack() as ctx:
        const = ctx.enter_context(tc.tile_pool(name="const", bufs=1))
        a_sb = const.tile([P, 2, KT, 1024], FP8)
        x1_sb = const.tile([P, KT, BL * H], FP8)
        x2_sb = const.tile([P, KT, BL * H], FP8)
        axw = const.tile([P, BL, N], BF16)
        axw2 = const.tile([P, BL, N], BF16)
        hrm_sb = const.tile([P, NT, BL, 2 * H], BF16)
        sig_r = const.tile([P, NT * BL, 2 * H], BF16)
        sig_u = const.tile([P, NT * BL, 2 * H], BF16)   # becomes um = 1-u
        uh = const.tile([P, NT, BL, 2 * H], BF16)       # u * h gate term
        w1_sb = const.tile([P, 2, 2 * H], BF16)
        w2_sb = const.tile([P, 2, H], BF16)

        a_ap = a_d.ap()
        x1_r = x1_d.ap().rearrange("p (kt f) -> p kt f", f=BL * H)
        axe_ap = axe_d.ap()
        axo_ap = axo_d.ap()
        hrm_ap = hrm_d.ap()

        # Priority ring (sync): A r-half + x1 tail in pass-0 consumption
        # order, then A u-half.  Patches and hrm queue AFTER on the same
        # ring, so ring FIFO keeps the whole A window uncontended.  The
        # first x1 pair rides the scalar ring in parallel.
        nc.sync.dma_start(a_sb[:, 0, 0:2, :], a_ap[:, 0, 0:2, :])
        nc.scalar.dma_start(x1_sb[:, 0:2, :], x1_r[:, 0:2, :])
        nc.sync.dma_start(a_sb[:, 0, 2:5, :], a_ap[:, 0, 2:5, :])
        nc.sync.dma_start(x1_sb[:, 2:8, :], x1_r[:, 2:8, :])
        nc.sync.dma_start(a_sb[:, 0, 5:8, :], a_ap[:, 0, 5:8, :])
        nc.sync.dma_start(a_sb[:, 0, 8:12, :], a_ap[:, 0, 8:12, :])
        nc.sync.dma_start(x1_sb[:, 8:16, :], x1_r[:, 8:16, :])
        nc.sync.dma_start(a_sb[:, 0, 12:16, :], a_ap[:, 0, 12:16, :])
        nc.sync.dma_start(a_sb[:, 1, 0:8, :], a_ap[:, 1, 0:8, :])
        nc.sync.dma_start(a_sb[:, 1, 8:16, :], a_ap[:, 1, 8:16, :])
        nc.scalar.dma_start(w1_sb[:], w1_d.ap())
        nc.scalar.dma_start(w2_sb[:], w2_d.ap())

        pps = ctx.enter_context(tc.tile_pool(name="ps", bufs=1, space="PSUM"))
        cpool = ctx.enter_context(tc.tile_pool(name="c", bufs=4))
        gpool = ctx.enter_context(tc.tile_pool(name="g", bufs=4))

        def bigtile(name):
            return pps.tile([P, 512], F32, tag="big", bufs=8, name=name)

        def eng_copy(eng, dst, ps):
            if eng is nc.vector:
                nc.vector.tensor_copy(out=dst, in_=ps[:])
            else:
                nc.scalar.copy(dst, ps[:])

        def patch_g1(cols):
            # axin/ones rows for ALL 4 batch-pairs in 2 DMAs (waits on all
            # 4 passes' drains of that column half via Tile deps)
            nc.sync.dma_start(axw[64:66, 0:BL:2, cols], axe_ap[:, :, cols])
            nc.sync.dma_start(axw[62:64, 1:BL:2, cols], axo_ap[:, :, cols])

        def g1r_joint():
            # one tp-major pass: all 4 batch-pairs x 2 chunks of the r half
            ps = [bigtile(f"g1r{mf}c{c}") for mf in range(4) for c in range(2)]
            for tp in range(NT):
                for mf in range(4):
                    lhsT = x1_sb[:, 2 * tp : 2 * tp + 2, ts(mf, P)]
                    for c in range(2):
                        nc.tensor.matmul(
                            ps[2 * mf + c][:],
                            lhsT=lhsT,
                            rhs=a_sb[:, 0, 2 * tp : 2 * tp + 2, ts(c, 512)],
                            start=(tp == 0), stop=(tp == NT - 1),
                            perf_mode=DR,
                        )
            # even-dst on vector, odd-dst on scalar, mf-ascending, so the
            # slot the next pass needs (mf0 c0) frees after ONE copy each
            for mf in range(4):
                for c in range(2):
                    cols = ds(c * 512, 512)
                    eng_copy(nc.vector, axw[:, 2 * mf, cols], ps[2 * mf + c])
                    eng_copy(nc.scalar, axw[:, 2 * mf + 1, cols], ps[2 * mf + c])
            patch_g1(ds(0, 1024))

        def g1u_pass(mf, all_vector=False):
            # tp-major over the 2 u-half chunks; batches 2mf, 2mf+1
            ps = [bigtile(f"g1u{mf}c{c}") for c in range(2)]
            for tp in range(NT):
                lhsT = x1_sb[:, 2 * tp : 2 * tp + 2, ts(mf, P)]
                for c in range(2):
                    nc.tensor.matmul(
                        ps[c][:],
                        lhsT=lhsT,
                        rhs=a_sb[:, 1, 2 * tp : 2 * tp + 2, ts(c, 512)],
                        start=(tp == 0), stop=(tp == NT - 1),
                        perf_mode=DR,
                    )
            for c in range(2):
                cols = ds(1024 + c * 512, 512)
                eng = nc.vector if (c == 0 or all_vector) else nc.scalar
                eng_copy(eng, axw[:, 2 * mf, cols], ps[c])
                eng_copy(eng, axw[:, 2 * mf + 1, cols], ps[c])

        def g2_pass(mf, split_patch):
            # chunk-major; batches 2mf, 2mf+1; drains split vector/scalar
            # trail each chunk; axin rows patched per pass (column-split
            # when a W2 phase starts right after this pass).
            for c in range(CH):
                ps = bigtile(f"g2m{mf}c{c}")
                for tp in range(NT):
                    nc.tensor.matmul(
                        ps[:],
                        lhsT=x2_sb[:, 2 * tp : 2 * tp + 2, ts(mf, P)],
                        rhs=a_sb[:, c // 2, 2 * tp : 2 * tp + 2, ts(c % 2, 512)],
                        start=(tp == 0), stop=(tp == NT - 1),
                        perf_mode=DR,
                    )
                cols = ds(c * 512, 512)
                nc.vector.tensor_copy(out=axw2[:, 2 * mf, cols], in_=ps[:])
                nc.scalar.copy(axw2[:, 2 * mf + 1, cols], ps[:])
                if split_patch and c % 2 == 1:
                    half = ds((c // 2) * 1024, 1024)
                    nc.sync.dma_start(axw2[64:66, 2 * mf, half], axe_ap[:, mf, half])
                    nc.sync.dma_start(axw2[62:64, 2 * mf + 1, half], axo_ap[:, mf, half])
            if not split_patch:
                nc.sync.dma_start(axw2[64:66, 2 * mf, :], axe_ap[:, mf, :])
                nc.sync.dma_start(axw2[62:64, 2 * mf + 1, :], axo_ap[:, mf, :])

        def w1_mt(mt):
            # one 128-node group x 8 batches: two 1-bank psum units, each
            # 4 MMs + one [128,512] sigmoid half
            for h in (0, 1):
                pm = bigtile(f"pm{mt}h{h}")
                pm3 = pm[:].rearrange("p (i f) -> p i f", i=4)
                for i in range(4):
                    b = 4 * h + i
                    nc.tensor.matmul(
                        pm3[:, i, :],
                        lhsT=axw[:, b, ts(mt, P)],
                        rhs=w1_sb[:, b % 2, :],
                        start=(i == 0), stop=(i == 3),
                    )
                if mt < NT:
                    dst = sig_r[:, ds(mt * BL + 4 * h, 4), :]
                else:
                    dst = sig_u[:, ds((mt - NT) * BL + 4 * h, 4), :]
                nc.scalar.activation(dst, pm[:], SIG)
            if mt >= NT:
                t = mt - NT
                su = sig_u[:, ts(t, BL), :]
                nc.vector.tensor_mul(uh[:, t, :, :], su, hrm_sb[:, t, :, :])
                nc.vector.tensor_scalar(
                    out=su, in0=su, scalar1=-1.0, scalar2=1.0,
                    op0=mybir.AluOpType.mult, op1=mybir.AluOpType.add,
                )

        def x2m(mt):
            # x2 assembly for r-group mt, on the otherwise-idle gpsimd so
            # the vector queue never saturates at the W1u/GCN2 junction
            for kt in (mt, mt + NT):
                jo = 0 if kt < NT else 64
                s3 = sig_r[:, ts(mt, BL), jo : jo + 64]
                x13 = x1_sb[:, kt, :].rearrange("p (b h) -> p b h", h=H)
                x23 = x2_sb[:, kt, :].rearrange("p (b h) -> p b h", h=H)
                nc.gpsimd.tensor_mul(x23, s3, x13)

        def w2_unit(t, hb, g, gi):
            # 4 batches of node group t: 8 MMs + tanh + fused gate
            pc = bigtile(f"pc{t}h{hb}")
            pc3 = pc[:].rearrange("p (i f) -> p i f", i=4)
            ch = t // 2
            for i in range(4):
                b = hb + i
                for j in (0, 1):
                    lo = 512 * ch + 256 * (t % 2) + j
                    nc.tensor.matmul(
                        pc3[:, i, ds(64 * j, 64)],
                        lhsT=axw2[:, b, lo : lo + 255 : 2],
                        rhs=w2_sb[:, b % 2, :],
                        start=(i == 0 and j == 0), stop=(i == 3 and j == 1),
                    )
            cs = cpool.tile([P, 4, 2 * H], BF16, tag="c")
            nc.scalar.activation(cs[:], pc[:], TANH)
            nc.vector.tensor_mul(g[:, gi], sig_u[:, ds(t * BL + hb, 4), :], cs[:])
            nc.vector.tensor_add(g[:, gi], g[:, gi], uh[:, t, hb : hb + 4, :])

        def w2_phase(hb):
            # t-pairs share one g tile and one store DMA; the final pair of
            # the last phase stores per-unit so the tail DMA fires early
            last = NT - 2 if hb == 4 else NT
            for tt in range(0, last, 2):
                g = gpool.tile([P, 2, 4, 2 * H], BF16, tag="g")
                w2_unit(tt, hb, g, 0)
                w2_unit(tt + 1, hb, g, 1)
                deng = nc.gpsimd if tt % 4 == 0 else nc.sync
                deng.dma_start(out_ap[:, tt : tt + 2, hb // 4, :, :], g[:])
            if hb == 4:
                for t in (NT - 2, NT - 1):
                    g = gpool.tile([P, 2, 4, 2 * H], BF16, tag="g")
                    w2_unit(t, hb, g, 0)
                    deng = nc.gpsimd if t == NT - 2 else nc.sync
                    deng.dma_start(out_ap[:, t, hb // 4, :, :], g[:, 0])

        # ---- GCN1 r-half: one joint supply-rate-matched pass ----
        g1r_joint()
        # hrm queued behind the patch-r triggers on the sync ring
        nc.sync.dma_start(hrm_sb[:], hrm_ap)
        # ---- GCN1 u-half passes interleaved with W1 r-groups; x2
        # assembly trails its sigmoids by one block ----
        g1u_pass(0)
        w1_mt(0); w1_mt(1)
        g1u_pass(1)
        x2m(0); x2m(1)
        w1_mt(2); w1_mt(3)
        g1u_pass(2)
        x2m(2); x2m(3)
        w1_mt(4); w1_mt(5)
        w1_mt(6); w1_mt(7)
        g1u_pass(3, all_vector=True)
        patch_g1(ds(1024, 1024))
        x2m(4); x2m(5); x2m(6); x2m(7)
        # ---- W1 u-groups interleaved with GCN2 passes ----
        w1_mt(8)
        w1_mt(9)
        g2_pass(0, split_patch=False)
        w1_mt(10); w1_mt(11); w1_mt(12); w1_mt(13); w1_mt(14); w1_mt(15)
        g2_pass(1, split_patch=True)
        # ---- W2 batches 0:4 (axw2 of GCN2 p0,p1) ----
        w2_phase(0)
        g2_pass(2, split_patch=False)
        g2_pass(3, split_patch=True)
        # ---- W2 batches 4:8 ----
        w2_phase(4)

    nc.finalize()
    return nc


def _prep_inputs(input_tensor, hidden, adj, W1, b1, W2, b2):
    f32 = np.float32
    bf16 = ml_dtypes.bfloat16
    fp8 = ml_dtypes.float8_e4m3
    input_tensor = np.ascontiguousarray(input_tensor, f32)
    hidden = np.ascontiguousarray(hidden, f32)
    adj = np.ascontiguousarray(adj, f32)
    W1 = np.asarray(W1, f32); b1 = np.asarray(b1, f32)
    W2 = np.asarray(W2, f32); b2 = np.asarray(b2, f32)

    pi = np.concatenate([np.arange(0, N, 2), np.arange(1, N, 2)])
    a_hat = adj + np.eye(N, dtype=f32)
    deg = a_hat.sum(axis=1, dtype=np.float64)
    d = (deg ** -0.5).astype(f32)

    # A columns scaled by 32*d[m], rows permuted; column-half-major layout
    a_s = (a_hat[pi] * (32.0 * d)[None, :]).astype(fp8)
    a_pre = np.ascontiguousarray(
        a_s.reshape(KT, P, N).transpose(1, 0, 2)           # [P, KT, N]
           .reshape(P, KT, 2, 1024).transpose(0, 2, 1, 3)  # [P, 2, KT, 1024]
    )

    # host A@input: axin[b, m] = 2^10 * d[m] * sum_n a_hat[n,m] d[n] in[b,n]
    din = (d[None, :] * input_tensor).astype(f32)
    axin_s = (1024.0 * (din @ a_hat) * d[None, :]).astype(bf16)     # (B, N)

    w1e = np.zeros((P, 2 * H), bf16)
    w1e[0:64] = (W1[1:] / 1024.0).astype(bf16)
    w1e[64] = (W1[0] / 1024.0).astype(bf16)
    w1e[65] = b1.astype(bf16)
    w1o = np.zeros((P, 2 * H), bf16)
    w1o[62] = (W1[0] / 1024.0).astype(bf16); w1o[63] = b1.astype(bf16)
    w1o[64:128] = (W1[1:] / 1024.0).astype(bf16)
    w2e = np.zeros((P, H), bf16)
    w2e[0:64] = (W2[1:] / 1024.0).astype(bf16)
    w2e[64] = (W2[0] / 1024.0).astype(bf16)
    w2e[65] = b2.astype(bf16)
    w2o = np.zeros((P, H), bf16)
    w2o[62] = (W2[0] / 1024.0).astype(bf16); w2o[63] = b2.astype(bf16)
    w2o[64:128] = (W2[1:] / 1024.0).astype(bf16)
    w1p = np.ascontiguousarray(np.stack([w1e, w1o], axis=1))    # [P, 2, 2H]
    w2p = np.ascontiguousarray(np.stack([w2e, w2o], axis=1))    # [P, 2, H]

    dh = (32.0 * d[None, :, None] * hidden).astype(f32)             # (B, N, H)

    in_maps = []
    for c in range(NCORES):
        bs = slice(BL * c, BL * c + BL)
        x1n = dh[bs][:, pi, :].transpose(1, 0, 2).reshape(N, BL * H)
        x1 = np.ascontiguousarray(
            x1n.reshape(KT, P, BL * H).transpose(1, 0, 2).reshape(P, KT * BL * H)
        ).astype(fp8)
        hrm = np.ascontiguousarray(
            hidden[bs].reshape(BL, NT, P, 2 * H).transpose(2, 1, 0, 3)
        ).astype(bf16)                                  # [P, NT, BL, 2H]
        axc = axin_s[bs]                                            # (8, N) bf16
        axe = np.zeros((2, 4, N), bf16)
        axe[0] = axc[0:BL:2]; axe[1] = 1.0
        axo = np.zeros((2, 4, N), bf16)
        axo[0] = axc[1:BL:2]; axo[1] = 1.0
        in_maps.append({
            "a": a_pre, "x1": x1, "hrm": hrm,
            "axe": axe, "axo": axo, "w1": w1p, "w2": w2p,
        })
    return in_maps


LAST_RESULTS = None


def kernel(input_tensor, hidden, adj, W1, b1, W2, b2):
    global LAST_RESULTS
    if "nc" not in _CACHE:
        _CACHE["nc"] = _build()
    nc = _CACHE["nc"]
    in_maps = _prep_inputs(input_tensor, hidden, adj, W1, b1, W2, b2)
    res = run_bass_kernel_spmd(nc, in_maps, core_ids=list(range(NCORES)))
    LAST_RESULTS = res
    outs = []
    for r in res.results:
        o = np.asarray(r["out"]).astype(np.float32)     # [P, NT, 2, 4, 2H]
        o = o.transpose(2, 3, 1, 0, 4).reshape(BL, NT * P, 2 * H)
        outs.append(o)
    return np.concatenate(outs, axis=0).reshape(B, N, H)


if __name__ == "__main__":
    rng = np.random.default_rng(0)
    inputs = {
        "input_tensor": rng.standard_normal((B, N), dtype=np.float32),
        "hidden": rng.standard_normal((B, N, H), dtype=np.float32),
        "adj": rng.random((N, N), dtype=np.float32),
        "W1": rng.standard_normal((H + 1, 2 * H), dtype=np.float32) * 0.15,
        "b1": np.full((2 * H,), 0.4, np.float32),
        "W2": rng.standard_normal((H + 1, H), dtype=np.float32) * 0.15,
        "b2": np.full((H,), 0.6, np.float32),
    }
    out = kernel(**inputs)
    print(out.shape, out.dtype)


# revision 23
# speedup vs baseline: 1.0552x; 1.0552x over previous
"""GCN-GRU cell fused Trainium2 kernel (8-core data parallel), v3.

Math (per batch b):
    A = d * (adj+I).T * d,  d = rowsum(adj+I)^-0.5
    conc1 = [input, hidden]                (N, 65)
    sig   = sigmoid(A @ conc1 @ W1 + b1)   (N, 128)  node-major flat
    r, u  = first/second half of flat(sig) -> pseudo-node split
    rh    = r * hidden_flat
    c     = tanh(A @ [input, rh] @ W2 + b2)
    out   = u * hidden_flat + (1-u) * c

v3 structure:
  - r/u pseudo-node split -> GCN1 runs in column halves: the r-half
    (A cols 0:1024) feeds W1 r-groups + x2 assembly while the u-half
    passes and GCN2 keep the PE busy; sigmoid latency never paces PE.
  - GCN1-r is ONE joint tp-major pass over all 4 batch-pairs x 2 chunks
    (8 psum banks live): A is consumed at ~supply rate, so the initial
    HBM-limited window has no PE stalls once started.
  - Single psum tag "big" = 8 x 1-bank ring for big chunks, W1 halves
    and W2 units: 8-deep reuse slack decouples activation pacing.
  - Full-width [128] psum drains write BOTH batch planes (garbage rows
    are nulled by zero rows in the packed W tiles); axin/ones rows are
    patched by tiny 2-row DMAs (removes v1's ~4MB zero-padding DMA).
  - One priority DMA ring (sync) carries A halves + x1 in consumption
    order, then patches, then hrm - ring FIFO guarantees A is never
    contended during the critical first ~19us (DMA pool saturates at
    ~283 GB/s aggregate).
  - GCN2 chunk-major with per-column-half patches; W2 in batch-halves
    (hb0 between GCN2 p1/p2, hb1 at the end) for a short store tail.
"""

import numpy as np
import ml_dtypes
from contextlib import ExitStack

import concourse.bacc as bacc
import concourse.mybir as mybir
import concourse.tile as tile
from concourse.bass import ts, ds
from concourse.bass_utils import run_bass_kernel_spmd

P = 128
N = 2048
B = 64
H = 64
NCORES = 8
BL = B // NCORES          # 8 batches per core
KT = N // P               # 16 contraction tiles
NT = KT // 2              # 8 (pair-tiles / half-node groups)
CH = N // 512             # 4 output chunks of 512
F32 = mybir.dt.float32
BF16 = mybir.dt.bfloat16
FP8 = mybir.dt.float8e4
SIG = mybir.ActivationFunctionType.Sigmoid
TANH = mybir.ActivationFunctionType.Tanh
DR = mybir.MatmulPerfMode.DoubleRow

_CACHE = {}


def _build():
    nc = bacc.Bacc("TRN2", target_bir_lowering=False)

    a_d = nc.dram_tensor("a", [P, 2, KT, 1024], FP8, kind="ExternalInput")
    x1_d = nc.dram_tensor("x1", [P, KT * BL * H], FP8, kind="ExternalInput")
    hrm_d = nc.dram_tensor("hrm", [P, NT, BL, 2 * H], BF16, kind="ExternalInput")
    axe_d = nc.dram_tensor("axe", [2, 4, N], BF16, kind="ExternalInput")
    axo_d = nc.dram_tensor("axo", [2, 4, N], BF16, kind="ExternalInput")
    w1_d = nc.dram_tensor("w1", [P, 2, 2 * H], BF16, kind="ExternalInput")
    w2_d = nc.dram_tensor("w2", [P, 2, H], BF16, kind="ExternalInput")
    out_d = nc.dram_tensor("out", [P, NT, 2, 4, 2 * H], BF16, kind="ExternalOutput")

    out_ap = out_d.ap()

    with tile.TileContext(nc) as tc, ExitStack() as ctx:
        const = ctx.enter_context(tc.tile_pool(name="const", bufs=1))
        a_sb = const.tile([P, 2, KT, 1024], FP8)
        x1_sb = const.tile([P, KT, BL * H], FP8)
        x2_sb = const.tile([P, KT, BL * H], FP8)
        axw = const.tile([P, BL, N], BF16)
        axw2 = const.tile([P, BL, N], BF16)
        hrm_sb = const.tile([P, NT, BL, 2 * H], BF16)
        sig_r = const.tile([P, NT * BL, 2 * H], BF16)
        sig_u = const.tile([P, NT * BL, 2 * H], BF16)   # becomes um = 1-u
        uh = const.tile([P, NT, BL, 2 * H], BF16)       # u * h gate term
        w1_sb = const.tile([P, 2, 2 * H], BF16)
        w2_sb = const.tile([P, 2, H], BF16)

        a_ap = a_d.ap()
        x1_r = x1_d.ap().rearrange("p (kt f) -> p kt f", f=BL * H)
        axe_ap = axe_d.ap()
        axo_ap = axo_d.ap()
        hrm_ap = hrm_d.ap()

        # Priority ring (sync): A r-half + x1 tail in pass-0 consumption
        # order, then A u-half.  Patches and hrm queue AFTER on the same
        # ring, so ring FIFO keeps the whole A window uncontended.  The
        # first x1 pair rides the scalar ring in parallel.
        nc.sync.dma_start(a_sb[:, 0, 0:2, :], a_ap[:, 0, 0:2, :])
        nc.scalar.dma_start(x1_sb[:, 0:2, :], x1_r[:, 0:2, :])
        nc.sync.dma_start(a_sb[:, 0, 2:5, :], a_ap[:, 0, 2:5, :])
        nc.sync.dma_start(x1_sb[:, 2:8, :], x1_r[:, 2:8, :])
        nc.sync.dma_start(a_sb[:, 0, 5:8, :], a_ap[:, 0, 5:8, :])
        nc.sync.dma_start(a_sb[:, 0, 8:12, :], a_ap[:, 0, 8:12, :])
        nc.sync.dma_start(x1_sb[:, 8:16, :], x1_r[:, 8:16, :])
        nc.sync.dma_start(a_sb[:, 0, 12:16, :], a_ap[:, 0, 12:16, :])
        nc.sync.dma_start(a_sb[:, 1, 0:8, :], a_ap[:, 1, 0:8, :])
        nc.sync.dma_start(a_sb[:, 1, 8:16, :], a_ap[:, 1, 8:16, :])
        nc.scalar.dma_start(w1_sb[:], w1_d.ap())
        nc.scalar.dma_start(w2_sb[:], w2_d.ap())

        pps = ctx.enter_context(tc.tile_pool(name="ps", bufs=1, space="PSUM"))
        cpool = ctx.enter_context(tc.tile_pool(name="c", bufs=4))
        gpool = ctx.enter_context(tc.tile_pool(name="g", bufs=4))

        def bigtile(name):
            return pps.tile([P, 512], F32, tag="big", bufs=8, name=name)

        def eng_copy(eng, dst, ps):
            if eng is nc.vector:
                nc.vector.tensor_copy(out=dst, in_=ps[:])
            else:
                nc.scalar.copy(dst, ps[:])

        def patch_g1(cols):
            # axin/ones rows for ALL 4 batch-pairs in 2 DMAs (waits on all
            # 4 passes' drains of that column half via Tile deps)
            nc.sync.dma_start(axw[64:66, 0:BL:2, cols], axe_ap[:, :, cols])
            nc.sync.dma_start(axw[62:64, 1:BL:2, cols], axo_ap[:, :, cols])

        def g1r_joint():
            # one tp-major pass: all 4 batch-pairs x 2 chunks of the r half
            ps = [bigtile(f"g1r{mf}c{c}") for mf in range(4) for c in range(2)]
            for tp in range(NT):
                for mf in range(4):
                    lhsT = x1_sb[:, 2 * tp : 2 * tp + 2, ts(mf, P)]
                    for c in range(2):
                        nc.tensor.matmul(
                            ps[2 * mf + c][:],
                            lhsT=lhsT,
                            rhs=a_sb[:, 0, 2 * tp : 2 * tp + 2, ts(c, 512)],
                            start=(tp == 0), stop=(tp == NT - 1),
                            perf_mode=DR,
                        )
            # even-dst on vector, odd-dst on scalar, mf-ascending, so the
            # slot the next pass needs (mf0 c0) frees after ONE copy each
            for mf in range(4):
                for c in range(2):
                    cols = ds(c * 512, 512)
                    eng_copy(nc.vector, axw[:, 2 * mf, cols], ps[2 * mf + c])
                    eng_copy(nc.scalar, axw[:, 2 * mf + 1, cols], ps[2 * mf + c])
            patch_g1(ds(0, 1024))

        def g1u_pass(mf, all_vector=False):
            # tp-major over the 2 u-half chunks; batches 2mf, 2mf+1
            ps = [bigtile(f"g1u{mf}c{c}") for c in range(2)]
            for tp in range(NT):
                lhsT = x1_sb[:, 2 * tp : 2 * tp + 2, ts(mf, P)]
                for c in range(2):
                    nc.tensor.matmul(
                        ps[c][:],
                        lhsT=lhsT,
                        rhs=a_sb[:, 1, 2 * tp : 2 * tp + 2, ts(c, 512)],
                        start=(tp == 0), stop=(tp == NT - 1),
                        perf_mode=DR,
                    )
            for c in range(2):
                cols = ds(1024 + c * 512, 512)
                eng_copy(nc.vector, axw[:, 2 * mf, cols], ps[c])
                eng_copy(nc.scalar, axw[:, 2 * mf + 1, cols], ps[c])

        def g2_pass(mf, split_patch):
            # chunk-major; batches 2mf, 2mf+1; drains split vector/scalar
            # trail each chunk; axin rows patched per pass (column-split
            # when a W2 phase starts right after this pass).
            for c in range(CH):
                ps = bigtile(f"g2m{mf}c{c}")
                for tp in range(NT):
                    nc.tensor.matmul(
                        ps[:],
                        lhsT=x2_sb[:, 2 * tp : 2 * tp + 2, ts(mf, P)],
                        rhs=a_sb[:, c // 2, 2 * tp : 2 * tp + 2, ts(c % 2, 512)],
                        start=(tp == 0), stop=(tp == NT - 1),
                        perf_mode=DR,
                    )
                cols = ds(c * 512, 512)
                nc.vector.tensor_copy(out=axw2[:, 2 * mf, cols], in_=ps[:])
                nc.scalar.copy(axw2[:, 2 * mf + 1, cols], ps[:])
                if split_patch and c % 2 == 1:
                    half = ds((c // 2) * 1024, 1024)
                    nc.sync.dma_start(axw2[64:66, 2 * mf, half], axe_ap[:, mf, half])
                    nc.sync.dma_start(axw2[62:64, 2 * mf + 1, half], axo_ap[:, mf, half])
            if not split_patch:
                nc.sync.dma_start(axw2[64:66, 2 * mf, :], axe_ap[:, mf, :])
                nc.sync.dma_start(axw2[62:64, 2 * mf + 1, :], axo_ap[:, mf, :])

        def w1_mt(mt):
            # one 128-node group x 8 batches: two 1-bank psum units, each
            # 4 MMs + one [128,512] sigmoid half
            for h in (0, 1):
                pm = bigtile(f"pm{mt}h{h}")
                pm3 = pm[:].rearrange("p (i f) -> p i f", i=4)
                for i in range(4):
                    b = 4 * h + i
                    nc.tensor.matmul(
                        pm3[:, i, :],
                        lhsT=axw[:, b, ts(mt, P)],
                        rhs=w1_sb[:, b % 2, :],
                        start=(i == 0), stop=(i == 3),
                    )
                if mt < NT:
                    dst = sig_r[:, ds(mt * BL + 4 * h, 4), :]
                else:
                    dst = sig_u[:, ds((mt - NT) * BL + 4 * h, 4), :]
                nc.scalar.activation(dst, pm[:], SIG)
            if mt >= NT:
                t = mt - NT
                su = sig_u[:, ts(t, BL), :]
                nc.vector.tensor_mul(uh[:, t, :, :], su, hrm_sb[:, t, :, :])
                nc.vector.tensor_scalar(
                    out=su, in0=su, scalar1=-1.0, scalar2=1.0,
                    op0=mybir.AluOpType.mult, op1=mybir.AluOpType.add,
                )

        def x2m(mt):
            # x2 assembly for r-group mt, on the otherwise-idle gpsimd so
            # the vector queue never saturates at the W1u/GCN2 junction
            for kt in (mt, mt + NT):
                jo = 0 if kt < NT else 64
                s3 = sig_r[:, ts(mt, BL), jo : jo + 64]
                x13 = x1_sb[:, kt, :].rearrange("p (b h) -> p b h", h=H)
                x23 = x2_sb[:, kt, :].rearrange("p (b h) -> p b h", h=H)
                nc.vector.tensor_mul(x23, s3, x13)

        def w2_unit(t, hb, g, gi):
            # 4 batches of node group t: 8 MMs + tanh + fused gate
            pc = bigtile(f"pc{t}h{hb}")
            pc3 = pc[:].rearrange("p (i f) -> p i f", i=4)
            ch = t // 2
            for i in range(4):
                b = hb + i
                for j in (0, 1):
                    lo = 512 * ch + 256 * (t % 2) + j
                    nc.tensor.matmul(
                        pc3[:, i, ds(64 * j, 64)],
                        lhsT=axw2[:, b, lo : lo + 255 : 2],
                        rhs=w2_sb[:, b % 2, :],
                        start=(i == 0 and j == 0), stop=(i == 3 and j == 1),
                    )
            cs = cpool.tile([P, 4, 2 * H], BF16, tag="c")
            nc.scalar.activation(cs[:], pc[:], TANH)
            nc.vector.tensor_mul(g[:, gi], sig_u[:, ds(t * BL + hb, 4), :], cs[:])
            nc.vector.tensor_add(g[:, gi], g[:, gi], uh[:, t, hb : hb + 4, :])

        def w2_phase(hb):
            # t-pairs share one g tile and one store DMA; the final pair of
            # the last phase stores per-unit so the tail DMA fires early
            last = NT - 2 if hb == 4 else NT
            for tt in range(0, last, 2):
                g = gpool.tile([P, 2, 4, 2 * H], BF16, tag="g")
                w2_unit(tt, hb, g, 0)
                w2_unit(tt + 1, hb, g, 1)
                deng = nc.gpsimd if tt % 4 == 0 else nc.sync
                deng.dma_start(out_ap[:, tt : tt + 2, hb // 4, :, :], g[:])
            if hb == 4:
                for t in (NT - 2, NT - 1):
                    g = gpool.tile([P, 2, 4, 2 * H], BF16, tag="g")
                    w2_unit(t, hb, g, 0)
                    deng = nc.gpsimd if t == NT - 2 else nc.sync
                    deng.dma_start(out_ap[:, t, hb // 4, :, :], g[:, 0])

        # ---- GCN1 r-half: one joint supply-rate-matched pass ----
        g1r_joint()
        # hrm queued behind the patch-r triggers on the sync ring
        nc.sync.dma_start(hrm_sb[:], hrm_ap)
        # ---- GCN1 u-half passes interleaved with W1 r-groups; x2
        # assembly trails its sigmoids by one block ----
        g1u_pass(0)
        w1_mt(0); w1_mt(1)
        g1u_pass(1)
        x2m(0); x2m(1)
        w1_mt(2); w1_mt(3)
        g1u_pass(2)
        x2m(2); x2m(3)
        w1_mt(4); w1_mt(5)
        x2m(4); x2m(5)
        w1_mt(6); w1_mt(7)
        g1u_pass(3)
        patch_g1(ds(1024, 1024))
        x2m(6); x2m(7)
        # ---- W1 u-groups interleaved with GCN2 passes ----
        w1_mt(8)
        w1_mt(9)
        g2_pass(0, split_patch=False)
        w1_mt(10); w1_mt(11); w1_mt(12); w1_mt(13); w1_mt(14); w1_mt(15)
        g2_pass(1, split_patch=True)
        # ---- W2 batches 0:4 (axw2 of GCN2 p0,p1) ----
        w2_phase(0)
        g2_pass(2, split_patch=False)
        g2_pass(3, split_patch=True)
        # ---- W2 batches 4:8 ----
        w2_phase(4)

    nc.finalize()
    return nc


def _prep_inputs(input_tensor, hidden, adj, W1, b1, W2, b2):
    f32 = np.float32
    bf16 = ml_dtypes.bfloat16
    fp8 = ml_dtypes.float8_e4m3
    input_tensor = np.ascontiguousarray(input_tensor, f32)
    hidden = np.ascontiguousarray(hidden, f32)
    adj = np.ascontiguousarray(adj, f32)
    W1 = np.asarray(W1, f32); b1 = np.asarray(b1, f32)
    W2 = np.asarray(W2, f32); b2 = np.asarray(b2, f32)

    pi = np.concatenate([np.arange(0, N, 2), np.arange(1, N, 2)])
    a_hat = adj + np.eye(N, dtype=f32)
    deg = a_hat.sum(axis=1, dtype=np.float64)
    d = (deg ** -0.5).astype(f32)

    # A columns scaled by 32*d[m], rows permuted; column-half-major layout
    a_s = (a_hat[pi] * (32.0 * d)[None, :]).astype(fp8)
    a_pre = np.ascontiguousarray(
        a_s.reshape(KT, P, N).transpose(1, 0, 2)           # [P, KT, N]
           .reshape(P, KT, 2, 1024).transpose(0, 2, 1, 3)  # [P, 2, KT, 1024]
    )

    # host A@input: axin[b, m] = 2^10 * d[m] * sum_n a_hat[n,m] d[n] in[b,n]
    din = (d[None, :] * input_tensor).astype(f32)
    axin_s = (1024.0 * (din @ a_hat) * d[None, :]).astype(bf16)     # (B, N)

    w1e = np.zeros((P, 2 * H), bf16)
    w1e[0:64] = (W1[1:] / 1024.0).astype(bf16)
    w1e[64] = (W1[0] / 1024.0).astype(bf16)
    w1e[65] = b1.astype(bf16)
    w1o = np.zeros((P, 2 * H), bf16)
    w1o[62] = (W1[0] / 1024.0).astype(bf16); w1o[63] = b1.astype(bf16)
    w1o[64:128] = (W1[1:] / 1024.0).astype(bf16)
    w2e = np.zeros((P, H), bf16)
    w2e[0:64] = (W2[1:] / 1024.0).astype(bf16)
    w2e[64] = (W2[0] / 1024.0).astype(bf16)
    w2e[65] = b2.astype(bf16)
    w2o = np.zeros((P, H), bf16)
    w2o[62] = (W2[0] / 1024.0).astype(bf16); w2o[63] = b2.astype(bf16)
    w2o[64:128] = (W2[1:] / 1024.0).astype(bf16)
    w1p = np.ascontiguousarray(np.stack([w1e, w1o], axis=1))    # [P, 2, 2H]
    w2p = np.ascontiguousarray(np.stack([w2e, w2o], axis=1))    # [P, 2, H]

    dh = (32.0 * d[None, :, None] * hidden).astype(f32)             # (B, N, H)

    in_maps = []
    for c in range(NCORES):
        bs = slice(BL * c, BL * c + BL)
        x1n = dh[bs][:, pi, :].transpose(1, 0, 2).reshape(N, BL * H)
        x1 = np.ascontiguousarray(
            x1n.reshape(KT, P, BL * H).transpose(1, 0, 2).reshape(P, KT * BL * H)
        ).astype(fp8)
        hrm = np.ascontiguousarray(
            hidden[bs].reshape(BL, NT, P, 2 * H).transpose(2, 1, 0, 3)
        ).astype(bf16)                                  # [P, NT, BL, 2H]
        axc = axin_s[bs]                                            # (8, N) bf16
        axe = np.zeros((2, 4, N), bf16)
        axe[0] = axc[0:BL:2]; axe[1] = 1.0
        axo = np.zeros((2, 4, N), bf16)
        axo[0] = axc[1:BL:2]; axo[1] = 1.0
        in_maps.append({
            "a": a_pre, "x1": x1, "hrm": hrm,
            "axe": axe, "axo": axo, "w1": w1p, "w2": w2p,
        })
    return in_maps


LAST_RESULTS = None


def kernel(input_tensor, hidden, adj, W1, b1, W2, b2):
    global LAST_RESULTS
    if "nc" not in _CACHE:
        _CACHE["nc"] = _build()
    nc = _CACHE["nc"]
    in_maps = _prep_inputs(input_tensor, hidden, adj, W1, b1, W2, b2)
    res = run_bass_kernel_spmd(nc, in_maps, core_ids=list(range(NCORES)))
    LAST_RESULTS = res
    outs = []
    for r in res.results:
        o = np.asarray(r["out"]).astype(np.float32)     # [P, NT, 2, 4, 2H]
        o = o.transpose(2, 3, 1, 0, 4).reshape(BL, NT * P, 2 * H)
        outs.append(o)
    return np.concatenate(outs, axis=0).reshape(B, N, H)


if __name__ == "__main__":
    rng = np.random.default_rng(0)
    inputs = {
        "input_tensor": rng.standard_normal((B, N), dtype=np.float32),
        "hidden": rng.standard_normal((B, N, H), dtype=np.float32),
        "adj": rng.random((N, N), dtype=np.float32),
        "W1": rng.standard_normal((H + 1, 2 * H), dtype=np.float32) * 0.15,
        "b1": np.full((2 * H,), 0.4, np.float32),
        "W2": rng.standard_normal((H + 1, H), dtype=np.float32) * 0.15,
        "b2": np.full((H,), 0.6, np.float32),
    }
    out = kernel(**inputs)
    print(out.shape, out.dtype)


# revision 25
# speedup vs baseline: 1.0986x; 1.0411x over previous
"""GCN-GRU cell fused Trainium2 kernel (8-core data parallel), v3.

Math (per batch b):
    A = d * (adj+I).T * d,  d = rowsum(adj+I)^-0.5
    conc1 = [input, hidden]                (N, 65)
    sig   = sigmoid(A @ conc1 @ W1 + b1)   (N, 128)  node-major flat
    r, u  = first/second half of flat(sig) -> pseudo-node split
    rh    = r * hidden_flat
    c     = tanh(A @ [input, rh] @ W2 + b2)
    out   = u * hidden_flat + (1-u) * c

v3 structure:
  - r/u pseudo-node split -> GCN1 runs in column halves: the r-half
    (A cols 0:1024) feeds W1 r-groups + x2 assembly while the u-half
    passes and GCN2 keep the PE busy; sigmoid latency never paces PE.
  - GCN1-r is ONE joint tp-major pass over all 4 batch-pairs x 2 chunks
    (8 psum banks live): A is consumed at ~supply rate, so the initial
    HBM-limited window has no PE stalls once started.
  - Single psum tag "big" = 8 x 1-bank ring for big chunks, W1 halves
    and W2 units: 8-deep reuse slack decouples activation pacing.
  - Full-width [128] psum drains write BOTH batch planes (garbage rows
    are nulled by zero rows in the packed W tiles); axin/ones rows are
    patched by tiny 2-row DMAs (removes v1's ~4MB zero-padding DMA).
  - One priority DMA ring (sync) carries A halves + x1 in consumption
    order, then patches, then hrm - ring FIFO guarantees A is never
    contended during the critical first ~19us (DMA pool saturates at
    ~283 GB/s aggregate).
  - GCN2 chunk-major with per-column-half patches; W2 in batch-halves
    (hb0 between GCN2 p1/p2, hb1 at the end) for a short store tail.
"""

import numpy as np
import ml_dtypes
from contextlib import ExitStack

import concourse.bacc as bacc
import concourse.mybir as mybir
import concourse.tile as tile
from concourse.bass import ts, ds
from concourse.bass_utils import run_bass_kernel_spmd

P = 128
N = 2048
B = 64
H = 64
NCORES = 8
BL = B // NCORES          # 8 batches per core
KT = N // P               # 16 contraction tiles
NT = KT // 2              # 8 (pair-tiles / half-node groups)
CH = N // 512             # 4 output chunks of 512
F32 = mybir.dt.float32
BF16 = mybir.dt.bfloat16
FP8 = mybir.dt.float8e4
SIG = mybir.ActivationFunctionType.Sigmoid
TANH = mybir.ActivationFunctionType.Tanh
DR = mybir.MatmulPerfMode.DoubleRow

_CACHE = {}


def _build():
    nc = bacc.Bacc("TRN2", target_bir_lowering=False)

    a_d = nc.dram_tensor("a", [P, 2, KT, 1024], FP8, kind="ExternalInput")
    x1_d = nc.dram_tensor("x1", [P, KT * BL * H], FP8, kind="ExternalInput")
    hrm_d = nc.dram_tensor("hrm", [P, NT, BL, 2 * H], BF16, kind="ExternalInput")
    axe_d = nc.dram_tensor("axe", [2, 4, N], BF16, kind="ExternalInput")
    axo_d = nc.dram_tensor("axo", [2, 4, N], BF16, kind="ExternalInput")
    w1_d = nc.dram_tensor("w1", [P, 2, 2 * H], BF16, kind="ExternalInput")
    w2_d = nc.dram_tensor("w2", [P, 2, H], BF16, kind="ExternalInput")
    out_d = nc.dram_tensor("out", [P, NT, 2, 4, 2 * H], BF16, kind="ExternalOutput")

    out_ap = out_d.ap()

    with tile.TileContext(nc) as tc, ExitStack() as ctx:
        const = ctx.enter_context(tc.tile_pool(name="const", bufs=1))
        a_sb = const.tile([P, 2, KT, 1024], FP8)
        x1_sb = const.tile([P, KT, BL * H], FP8)
        x2_sb = const.tile([P, KT, BL * H], FP8)
        axw = const.tile([P, BL, N], BF16)
        axw2 = const.tile([P, BL, N], BF16)
        hrm_sb = const.tile([P, NT, BL, 2 * H], BF16)
        sig_r = const.tile([P, NT * BL, 2 * H], BF16)
        sig_u = const.tile([P, NT * BL, 2 * H], BF16)   # becomes um = 1-u
        uh = const.tile([P, NT, BL, 2 * H], BF16)       # u * h gate term
        w1_sb = const.tile([P, 2, 2 * H], BF16)
        w2_sb = const.tile([P, 2, H], BF16)

        a_ap = a_d.ap()
        x1_r = x1_d.ap().rearrange("p (kt f) -> p kt f", f=BL * H)
        axe_ap = axe_d.ap()
        axo_ap = axo_d.ap()
        hrm_ap = hrm_d.ap()

        # Priority ring (sync): A r-half + x1 tail in pass-0 consumption
        # order, then A u-half.  Patches and hrm queue AFTER on the same
        # ring, so ring FIFO keeps the whole A window uncontended.  The
        # first x1 pair rides the scalar ring in parallel.
        nc.sync.dma_start(a_sb[:, 0, 0:2, :], a_ap[:, 0, 0:2, :])
        nc.scalar.dma_start(x1_sb[:, 0:2, :], x1_r[:, 0:2, :])
        nc.sync.dma_start(a_sb[:, 0, 2:5, :], a_ap[:, 0, 2:5, :])
        nc.sync.dma_start(x1_sb[:, 2:8, :], x1_r[:, 2:8, :])
        nc.sync.dma_start(a_sb[:, 0, 5:8, :], a_ap[:, 0, 5:8, :])
        nc.sync.dma_start(a_sb[:, 0, 8:12, :], a_ap[:, 0, 8:12, :])
        nc.sync.dma_start(x1_sb[:, 8:16, :], x1_r[:, 8:16, :])
        nc.sync.dma_start(a_sb[:, 0, 12:16, :], a_ap[:, 0, 12:16, :])
        nc.sync.dma_start(a_sb[:, 1, 0:8, :], a_ap[:, 1, 0:8, :])
        nc.sync.dma_start(a_sb[:, 1, 8:16, :], a_ap[:, 1, 8:16, :])
        nc.scalar.dma_start(w1_sb[:], w1_d.ap())
        nc.scalar.dma_start(w2_sb[:], w2_d.ap())

        pps = ctx.enter_context(tc.tile_pool(name="ps", bufs=1, space="PSUM"))
        cpool = ctx.enter_context(tc.tile_pool(name="c", bufs=4))
        gpool = ctx.enter_context(tc.tile_pool(name="g", bufs=4))

        def bigtile(name):
            return pps.tile([P, 512], F32, tag="big", bufs=8, name=name)

        def eng_copy(eng, dst, ps):
            if eng is nc.vector:
                nc.vector.tensor_copy(out=dst, in_=ps[:])
            else:
                nc.scalar.copy(dst, ps[:])

        def patch_g1(cols):
            # axin/ones rows for ALL 4 batch-pairs in 2 DMAs (waits on all
            # 4 passes' drains of that column half via Tile deps)
            nc.sync.dma_start(axw[64:66, 0:BL:2, cols], axe_ap[:, :, cols])
            nc.sync.dma_start(axw[62:64, 1:BL:2, cols], axo_ap[:, :, cols])

        def g1r_joint():
            # one tp-major pass: all 4 batch-pairs x 2 chunks of the r half
            ps = [bigtile(f"g1r{mf}c{c}") for mf in range(4) for c in range(2)]
            for tp in range(NT):
                for mf in range(4):
                    lhsT = x1_sb[:, 2 * tp : 2 * tp + 2, ts(mf, P)]
                    for c in range(2):
                        nc.tensor.matmul(
                            ps[2 * mf + c][:],
                            lhsT=lhsT,
                            rhs=a_sb[:, 0, 2 * tp : 2 * tp + 2, ts(c, 512)],
                            start=(tp == 0), stop=(tp == NT - 1),
                            perf_mode=DR,
                        )
            for mf in range(4):
                eng = nc.vector if mf % 2 == 0 else nc.scalar
                for c in range(2):
                    cols = ds(c * 512, 512)
                    eng_copy(eng, axw[:, 2 * mf, cols], ps[2 * mf + c])
                    eng_copy(eng, axw[:, 2 * mf + 1, cols], ps[2 * mf + c])
            patch_g1(ds(0, 1024))

        def g1u_pass(mf, all_vector=False):
            # tp-major over the 2 u-half chunks; batches 2mf, 2mf+1
            ps = [bigtile(f"g1u{mf}c{c}") for c in range(2)]
            for tp in range(NT):
                lhsT = x1_sb[:, 2 * tp : 2 * tp + 2, ts(mf, P)]
                for c in range(2):
                    nc.tensor.matmul(
                        ps[c][:],
                        lhsT=lhsT,
                        rhs=a_sb[:, 1, 2 * tp : 2 * tp + 2, ts(c, 512)],
                        start=(tp == 0), stop=(tp == NT - 1),
                        perf_mode=DR,
                    )
            for c in range(2):
                cols = ds(1024 + c * 512, 512)
                eng = nc.vector if c == 0 else nc.scalar
                eng_copy(eng, axw[:, 2 * mf, cols], ps[c])
                eng_copy(eng, axw[:, 2 * mf + 1, cols], ps[c])

        def g2_pass(mf, split_patch):
            # chunk-major; batches 2mf, 2mf+1; drains split vector/scalar
            # trail each chunk; axin rows patched per pass (column-split
            # when a W2 phase starts right after this pass).
            for c in range(CH):
                ps = bigtile(f"g2m{mf}c{c}")
                for tp in range(NT):
                    nc.tensor.matmul(
                        ps[:],
                        lhsT=x2_sb[:, 2 * tp : 2 * tp + 2, ts(mf, P)],
                        rhs=a_sb[:, c // 2, 2 * tp : 2 * tp + 2, ts(c % 2, 512)],
                        start=(tp == 0), stop=(tp == NT - 1),
                        perf_mode=DR,
                    )
                cols = ds(c * 512, 512)
                nc.vector.tensor_copy(out=axw2[:, 2 * mf, cols], in_=ps[:])
                nc.scalar.copy(axw2[:, 2 * mf + 1, cols], ps[:])
                if split_patch and c % 2 == 1:
                    half = ds((c // 2) * 1024, 1024)
                    nc.sync.dma_start(axw2[64:66, 2 * mf, half], axe_ap[:, mf, half])
                    nc.sync.dma_start(axw2[62:64, 2 * mf + 1, half], axo_ap[:, mf, half])
            if not split_patch:
                nc.sync.dma_start(axw2[64:66, 2 * mf, :], axe_ap[:, mf, :])
                nc.sync.dma_start(axw2[62:64, 2 * mf + 1, :], axo_ap[:, mf, :])

        def w1_mt(mt):
            # one 128-node group x 8 batches: two 1-bank psum units, each
            # 4 MMs + one [128,512] sigmoid half
            for h in (0, 1):
                pm = bigtile(f"pm{mt}h{h}")
                pm3 = pm[:].rearrange("p (i f) -> p i f", i=4)
                for i in range(4):
                    b = 4 * h + i
                    nc.tensor.matmul(
                        pm3[:, i, :],
                        lhsT=axw[:, b, ts(mt, P)],
                        rhs=w1_sb[:, b % 2, :],
                        start=(i == 0), stop=(i == 3),
                    )
                if mt < NT:
                    dst = sig_r[:, ds(mt * BL + 4 * h, 4), :]
                else:
                    dst = sig_u[:, ds((mt - NT) * BL + 4 * h, 4), :]
                nc.scalar.activation(dst, pm[:], SIG)
            if mt >= NT:
                t = mt - NT
                su = sig_u[:, ts(t, BL), :]
                nc.vector.tensor_mul(uh[:, t, :, :], su, hrm_sb[:, t, :, :])
                nc.vector.tensor_scalar(
                    out=su, in0=su, scalar1=-1.0, scalar2=1.0,
                    op0=mybir.AluOpType.mult, op1=mybir.AluOpType.add,
                )

        def x2m(mt):
            # x2 assembly for r-group mt, on the otherwise-idle gpsimd so
            # the vector queue never saturates at the W1u/GCN2 junction
            for kt in (mt, mt + NT):
                jo = 0 if kt < NT else 64
                s3 = sig_r[:, ts(mt, BL), jo : jo + 64]
                x13 = x1_sb[:, kt, :].rearrange("p (b h) -> p b h", h=H)
                x23 = x2_sb[:, kt, :].rearrange("p (b h) -> p b h", h=H)
                nc.vector.tensor_mul(x23, s3, x13)

        def w2_unit(t, hb, g, gi):
            # 4 batches of node group t: 8 MMs + tanh + fused gate
            pc = bigtile(f"pc{t}h{hb}")
            pc3 = pc[:].rearrange("p (i f) -> p i f", i=4)
            ch = t // 2
            for i in range(4):
                b = hb + i
                for j in (0, 1):
                    lo = 512 * ch + 256 * (t % 2) + j
                    nc.tensor.matmul(
                        pc3[:, i, ds(64 * j, 64)],
                        lhsT=axw2[:, b, lo : lo + 255 : 2],
                        rhs=w2_sb[:, b % 2, :],
                        start=(i == 0 and j == 0), stop=(i == 3 and j == 1),
                    )
            cs = cpool.tile([P, 4, 2 * H], BF16, tag="c")
            nc.scalar.activation(cs[:], pc[:], TANH)
            nc.vector.tensor_mul(g[:, gi], sig_u[:, ds(t * BL + hb, 4), :], cs[:])
            nc.vector.tensor_add(g[:, gi], g[:, gi], uh[:, t, hb : hb + 4, :])

        def w2_phase(hb):
            # t-pairs share one g tile and one store DMA; the final pair of
            # the last phase stores per-unit so the tail DMA fires early
            last = NT - 2 if hb == 4 else NT
            for tt in range(0, last, 2):
                g = gpool.tile([P, 2, 4, 2 * H], BF16, tag="g")
                w2_unit(tt, hb, g, 0)
                w2_unit(tt + 1, hb, g, 1)
                deng = nc.gpsimd if tt % 4 == 0 else nc.sync
                deng.dma_start(out_ap[:, tt : tt + 2, hb // 4, :, :], g[:])
            if hb == 4:
                for t in (NT - 2, NT - 1):
                    g = gpool.tile([P, 2, 4, 2 * H], BF16, tag="g")
                    w2_unit(t, hb, g, 0)
                    deng = nc.gpsimd if t == NT - 2 else nc.sync
                    deng.dma_start(out_ap[:, t, hb // 4, :, :], g[:, 0])

        # ---- GCN1 r-half: one joint supply-rate-matched pass ----
        g1r_joint()
        # hrm queued behind the patch-r triggers on the sync ring
        nc.sync.dma_start(hrm_sb[:], hrm_ap)
        # ---- GCN1 u-half passes interleaved with W1 r-groups; x2
        # assembly trails its sigmoids by one block ----
        g1u_pass(0)
        w1_mt(0); w1_mt(1)
        g1u_pass(1)
        x2m(0); x2m(1)
        w1_mt(2); w1_mt(3)
        g1u_pass(2)
        x2m(2); x2m(3)
        w1_mt(4); w1_mt(5)
        x2m(4); x2m(5)
        w1_mt(6); w1_mt(7)
        g1u_pass(3)
        patch_g1(ds(1024, 1024))
        x2m(6); x2m(7)
        # ---- W1 u-groups interleaved with GCN2 passes ----
        w1_mt(8)
        w1_mt(9)
        g2_pass(0, split_patch=False)
        w1_mt(10); w1_mt(11); w1_mt(12); w1_mt(13); w1_mt(14); w1_mt(15)
        g2_pass(1, split_patch=True)
        # ---- W2 batches 0:4 (axw2 of GCN2 p0,p1) ----
        w2_phase(0)
        g2_pass(2, split_patch=False)
        g2_pass(3, split_patch=True)
        # ---- W2 batches 4:8 ----
        w2_phase(4)

    nc.finalize()
    return nc


def _prep_inputs(input_tensor, hidden, adj, W1, b1, W2, b2):
    f32 = np.float32
    bf16 = ml_dtypes.bfloat16
    fp8 = ml_dtypes.float8_e4m3
    input_tensor = np.ascontiguousarray(input_tensor, f32)
    hidden = np.ascontiguousarray(hidden, f32)
    adj = np.ascontiguousarray(adj, f32)
    W1 = np.asarray(W1, f32); b1 = np.asarray(b1, f32)
    W2 = np.asarray(W2, f32); b2 = np.asarray(b2, f32)

    pi = np.concatenate([np.arange(0, N, 2), np.arange(1, N, 2)])
    a_hat = adj + np.eye(N, dtype=f32)
    deg = a_hat.sum(axis=1, dtype=np.float64)
    d = (deg ** -0.5).astype(f32)

    # A columns scaled by 32*d[m], rows permuted; column-half-major layout
    a_s = (a_hat[pi] * (32.0 * d)[None, :]).astype(fp8)
    a_pre = np.ascontiguousarray(
        a_s.reshape(KT, P, N).transpose(1, 0, 2)           # [P, KT, N]
           .reshape(P, KT, 2, 1024).transpose(0, 2, 1, 3)  # [P, 2, KT, 1024]
    )

    # host A@input: axin[b, m] = 2^10 * d[m] * sum_n a_hat[n,m] d[n] in[b,n]
    din = (d[None, :] * input_tensor).astype(f32)
    axin_s = (1024.0 * (din @ a_hat) * d[None, :]).astype(bf16)     # (B, N)

    w1e = np.zeros((P, 2 * H), bf16)
    w1e[0:64] = (W1[1:] / 1024.0).astype(bf16)
    w1e[64] = (W1[0] / 1024.0).astype(bf16)
    w1e[65] = b1.astype(bf16)
    w1o = np.zeros((P, 2 * H), bf16)
    w1o[62] = (W1[0] / 1024.0).astype(bf16); w1o[63] = b1.astype(bf16)
    w1o[64:128] = (W1[1:] / 1024.0).astype(bf16)
    w2e = np.zeros((P, H), bf16)
    w2e[0:64] = (W2[1:] / 1024.0).astype(bf16)
    w2e[64] = (W2[0] / 1024.0).astype(bf16)
    w2e[65] = b2.astype(bf16)
    w2o = np.zeros((P, H), bf16)
    w2o[62] = (W2[0] / 1024.0).astype(bf16); w2o[63] = b2.astype(bf16)
    w2o[64:128] = (W2[1:] / 1024.0).astype(bf16)
    w1p = np.ascontiguousarray(np.stack([w1e, w1o], axis=1))    # [P, 2, 2H]
    w2p = np.ascontiguousarray(np.stack([w2e, w2o], axis=1))    # [P, 2, H]

    dh = (32.0 * d[None, :, None] * hidden).astype(f32)             # (B, N, H)

    in_maps = []
    for c in range(NCORES):
        bs = slice(BL * c, BL * c + BL)
        x1n = dh[bs][:, pi, :].transpose(1, 0, 2).reshape(N, BL * H)
        x1 = np.ascontiguousarray(
            x1n.reshape(KT, P, BL * H).transpose(1, 0, 2).reshape(P, KT * BL * H)
        ).astype(fp8)
        hrm = np.ascontiguousarray(
            hidden[bs].reshape(BL, NT, P, 2 * H).transpose(2, 1, 0, 3)
        ).astype(bf16)                                  # [P, NT, BL, 2H]
        axc = axin_s[bs]                                            # (8, N) bf16
        axe = np.zeros((2, 4, N), bf16)
        axe[0] = axc[0:BL:2]; axe[1] = 1.0
        axo = np.zeros((2, 4, N), bf16)
        axo[0] = axc[1:BL:2]; axo[1] = 1.0
        in_maps.append({
            "a": a_pre, "x1": x1, "hrm": hrm,
            "axe": axe, "axo": axo, "w1": w1p, "w2": w2p,
        })
    return in_maps


LAST_RESULTS = None


def kernel(input_tensor, hidden, adj, W1, b1, W2, b2):
    global LAST_RESULTS
    if "nc" not in _CACHE:
        _CACHE["nc"] = _build()
    nc = _CACHE["nc"]
    in_maps = _prep_inputs(input_tensor, hidden, adj, W1, b1, W2, b2)
    res = run_bass_kernel_spmd(nc, in_maps, core_ids=list(range(NCORES)))
    LAST_RESULTS = res
    outs = []
    for r in res.results:
        o = np.asarray(r["out"]).astype(np.float32)     # [P, NT, 2, 4, 2H]
        o = o.transpose(2, 3, 1, 0, 4).reshape(BL, NT * P, 2 * H)
        outs.append(o)
    return np.concatenate(outs, axis=0).reshape(B, N, H)


if __name__ == "__main__":
    rng = np.random.default_rng(0)
    inputs = {
        "input_tensor": rng.standard_normal((B, N), dtype=np.float32),
        "hidden": rng.standard_normal((B, N, H), dtype=np.float32),
        "adj": rng.random((N, N), dtype=np.float32),
        "W1": rng.standard_normal((H + 1, 2 * H), dtype=np.float32) * 0.15,
        "b1": np.full((2 * H,), 0.4, np.float32),
        "W2": rng.standard_normal((H + 1, H), dtype=np.float32) * 0.15,
        "b2": np.full((H,), 0.6, np.float32),
    }
    out = kernel(**inputs)
    print(out.shape, out.dtype)
